# revision 40
# baseline (speedup 1.0000x reference)
"""Trainium2 Bass kernel for a DGCNN-style point-cloud encoder.

Per batch element (one per NeuronCore, B=8): kNN graph (k=20) over N=4096
points via a distance matmul + chunked top-k on packed f16 scores, edge-
feature MLP with two training-mode batchnorms (global stats via cross-core
AllReduce) and leaky-relu, then max-pool over neighbors.

Key layout choices:
- scores are stored PACKED as u32 words (f16 score bits << 16 | column idx)
  in SBUF; the iota column indices are pre-filled once per rotating buffer,
  the ACT engine writes f16 scores (bias -1/64 keeps every word out of the
  f32-denormal range) into the strided hi-u16 slots. One DVE max8 pass per
  256-column chunk then yields top-8 (value|index) candidates directly --
  no FIND_INDEX8 pass and no index-base fixup.
- three tiny max8/match_replace rounds merge the 128 candidates into the
  top-20 with exact smaller-index tie-breaking.
- gather: the 64-channel A-table (A = W1a @ x) is kept in SBUF as
  [128, 4096] u32 (value f16 in the lo half, channels duplicated across
  partition halves).  nc.gpsimd.ap_gather pulls neighbor columns straight
  out of SBUF through the Q7 TIE queues -- no SWDGE DMA descriptors, which
  were the previous bottleneck (1.7ms of Q7 descriptor generation).  Cores
  0-3 gather the even-rank neighbors, cores 4-7 the odd ranks, using
  per-core index lists built by one scatter/gather DMA pair through DRAM.
- W2 is block-diagonal [128,128] so one matmul computes both neighbor
  parities; max-pool and BN2 stats read the parity-stacked PSUM at full
  128-lane width.
"""
import sys
sys.path.insert(0, '/opt/trn_rl_repo')

import numpy as np
import orjson

import concourse.bass as bass
import concourse.mybir as mybir
import concourse.tile as tile
from concourse import library_config
from concourse.bass_utils import run_bass_kernel_spmd

# ---------------------------------------------------------------------------
# Workaround for walrus 'Too many sync wait commands': this toolchain accepts
# at most one sem-wait per lowered instruction. Split any instruction carrying
# more waits into EventSemaphore wait-carriers placed immediately before it.
# ---------------------------------------------------------------------------
_MAXW = 1


def _split_excess_waits(j) -> bool:
    changed = False
    for fn in j.get("functions", []):
        for blk in fn.get("blocks", []):
            out = []
            for inst in blk.get("instructions", []):
                si = inst.get("sync_info") or {}
                ow = si.get("on_wait") or []
                if len(ow) > _MAXW:
                    changed = True
                    chunks = [ow[i:i + _MAXW] for i in range(0, len(ow), _MAXW)]
                    for ci, chunk in enumerate(chunks[:-1]):
                        out.append({
                            "debug": inst.get("debug", 0),
                            "engine": inst["engine"],
                            "ins": [], "outs": [],
                            "name": f"{inst['name']}-w{ci}",
                            "opcode": "EventSemaphore",
                            "sync_info": {"on_update": [], "on_wait": chunk},
                        })
                    si = dict(si)
                    si["on_wait"] = chunks[-1]
                    inst = dict(inst)
                    inst["sync_info"] = si
                out.append(inst)
            blk["instructions"] = out
    return changed


_orig_to_json_bytes = bass.Bass.to_json_bytes


def _patched_to_json_bytes(self) -> bytes:
    raw = _orig_to_json_bytes(self)
    j = orjson.loads(raw)
    if _split_excess_waits(j):
        return orjson.dumps(j)
    return raw


bass.Bass.to_json_bytes = _patched_to_json_bytes

# ---------------------------------------------------------------------------
# Problem constants (hardcoded; kernel.py must be self-contained)
# ---------------------------------------------------------------------------
B = 8            # batch = number of cores
N = 4096         # points per cloud
KNN = 20         # neighbors
CH = 64          # hidden channels
EPS = 1e-5
ALPHA = 0.2      # leaky-relu slope
NM = N // 128    # 32 row-tiles
NJ2 = KNN // 2   # 10 neighbor pairs per parity
CNT = B * N * KNN  # batchnorm population size (global over all cores)
GNI = NJ2 * 128  # gathered indices per parity per tile (1280)

f32 = mybir.dt.float32
f16 = mybir.dt.float16
u16 = mybir.dt.uint16
i16 = mybir.dt.int16
u32 = mybir.dt.uint32
ACTF = mybir.ActivationFunctionType
ALU = mybir.AluOpType


def _redim(ap, dims, extra_off=0):
    """Rebuild an AP keeping tensor/offset/partition dim, custom free dims."""
    return bass.AP(ap.tensor, ap.offset + extra_off, [list(ap.ap[0])] + dims)


def _build_program():
    nc = bass.Bass("TRN2", target_bir_lowering=False, debug=False,
                   num_devices=B)

    xb = nc.dram_tensor("xb", [4, N], f32, kind="ExternalInput")
    w1at2 = nc.dram_tensor("w1at2", [3, 128], f16, kind="ExternalInput")
    w1ct2 = nc.dram_tensor("w1ct2", [3, 128], f16, kind="ExternalInput")
    w2t = nc.dram_tensor("w2t", [128, 128], f16, kind="ExternalInput")
    iota_d = nc.dram_tensor("iota", [1, N], u32, kind="ExternalInput")
    bn1g = nc.dram_tensor("bn1g", [CH, 1], f32, kind="ExternalInput")
    bn1b = nc.dram_tensor("bn1b", [CH, 1], f32, kind="ExternalInput")
    bn2g = nc.dram_tensor("bn2g", [CH, 1], f32, kind="ExternalInput")
    bn2b = nc.dram_tensor("bn2b", [CH, 1], f32, kind="ExternalInput")
    out_t = nc.dram_tensor("out", [CH, N], f32, kind="ExternalOutput")

    cc1_in = nc.dram_tensor("cc1_in", [128, 2], f32)
    cc1_out = nc.dram_tensor("cc1_out", [128, 2], f32, addr_space="Shared")
    cc2_in = nc.dram_tensor("cc2_in", [CH, 2], f32)
    cc2_out = nc.dram_tensor("cc2_out", [CH, 2], f32, addr_space="Shared")
    groups = [list(range(B))]

    with tile.TileContext(nc) as tc:
        nc.gpsimd.load_library(library_config.ap_gather)

        const = tc.alloc_tile_pool(name="const", bufs=1)
        dramp = tc.alloc_tile_pool(name="dram", bufs=5, space="DRAM")
        abpool = tc.alloc_tile_pool(name="ab", bufs=1)

        # whole-kernel tensors
        w2t_sb = const.tile([128, 128], f16)
        g1_sb = const.tile([CH, 1], f32)
        b1in_sb = const.tile([CH, 1], f32)
        g2_sb = const.tile([CH, 1], f32)
        b2in_sb = const.tile([CH, 1], f32)
        h1p = const.tile([128, NM * GNI], f16)   # stored h1_pre
        a1_sb = const.tile([128, 1], f32)
        b1_sb = const.tile([128, 1], f32)
        a2_sb = const.tile([CH, 1], f32)
        b2_sb = const.tile([CH, 1], f32)

        # phase A/B tensors (released after phase AB)
        # K=15 concatenated split operands: [hi;hi;lo] x [hi;lo;hi] computes
        # hi*hi + hi*lo + lo*hi in ONE matmul (fp32 to ~2^-24 rel)
        lhs_cat = abpool.tile([15, N], f16)
        rhs_cat = abpool.tile([15, N], f16)
        w1at2_sb = abpool.tile([3, 128], f16)
        w1ct2_sb = abpool.tile([3, 128], f16)
        s1sum = abpool.tile([128, NM], f32)
        s1sq = abpool.tile([128, NM], f32)
        tab32 = abpool.tile([128, N], u32)     # A-table, f16 value in lo u16

        nc.sync.dma_start(w1at2_sb[:], w1at2.ap())
        nc.sync.dma_start(w1ct2_sb[:], w1ct2.ap())
        nc.sync.dma_start(w2t_sb[:], w2t.ap())
        nc.sync.dma_start(g1_sb[:], bn1g.ap())
        nc.sync.dma_start(b1in_sb[:], bn1b.ap())
        nc.sync.dma_start(g2_sb[:], bn2g.ap())
        nc.sync.dma_start(b2in_sb[:], bn2b.ap())

        with nc.named_scope("stage0"):
            with tc.tile_pool(name="s0", bufs=1) as s0big, \
                 tc.tile_pool(name="s0s", bufs=1) as s0pool, \
                 tc.tile_pool(name="s0ps", bufs=2, space="PSUM") as s0psum:
                lhs_all = s0big.tile([5, N], f32, tag="lhs")
                rhs_all = s0big.tile([5, N], f32, tag="rhs")
                nc.sync.dma_start(lhs_all[0:4, :], xb.ap())
                nc.scalar.mul(rhs_all[0:3, :], lhs_all[0:3, :], 2.0)
                negc = s0big.tile([1, 512], f32, tag="negc")
                nc.vector.memset(negc[:], -1.0)
                ng = negc[:]
                nc.sync.dma_start(
                    rhs_all[4:5, :],
                    bass.AP(ng.tensor, ng.offset, [list(ng.ap[0]),
                                                   [0, 8], [1, 512]]))
                xsq = s0big.tile([4, N], f32, tag="scratch")
                nc.scalar.square(xsq[0:3, :], lhs_all[0:3, :])
                ones3 = nc.const_aps.tensor(1.0, (3, 1), f32)
                for j in range(N // 512):
                    ps = s0psum.tile([1, 512], f32, space="PSUM", tag="sq")
                    nc.tensor.matmul(ps[:], lhsT=ones3,
                                     rhs=xsq[0:3, bass.ts(j, 512)],
                                     start=True, stop=True)
                    sqpos = s0pool.tile([1, 512], f32, tag="sqpos")
                    nc.scalar.copy(sqpos[:], ps[:])
                    nc.sync.dma_start(lhs_all[4:5, bass.ts(j, 512)], sqpos[:])
                    sqneg = s0pool.tile([1, 512], f32, tag="sqneg")
                    nc.scalar.activation(sqneg[:], ps[:], ACTF.Copy, scale=-1.0)
                    nc.sync.dma_start(rhs_all[3:4, bass.ts(j, 512)], sqneg[:])
                # fp16 hi/lo split of lhs/rhs for single-pass PE matmuls
                for full, cat, pattern in ((lhs_all, lhs_cat, (0, 0, 1)),
                                           (rhs_all, rhs_cat, (0, 1, 0))):
                    hi = s0pool.tile([5, N], f16, tag="hi")
                    nc.scalar.copy(hi[:], full[:])
                    rb = s0big.tile([5, N], f32, tag="scratch")
                    nc.vector.tensor_sub(rb[:], full[:], hi[:])
                    lo = s0pool.tile([5, N], f16, tag="lo")
                    nc.scalar.copy(lo[:], rb[:])
                    for slot, which in enumerate(pattern):
                        nc.sync.dma_start(cat[slot * 5:(slot + 1) * 5, :],
                                          (hi if which == 0 else lo)[:])

        # ------------------------------------------------------------------
        # Phases A (scores + top-k) and B (gather, C add, BN1 stats)
        # ------------------------------------------------------------------
        with nc.named_scope("phaseAB"), \
             tc.tile_pool(name="scps", bufs=2, space="PSUM") as scps, \
             tc.tile_pool(name="ctps", bufs=2, space="PSUM") as ctps, \
             tc.tile_pool(name="score", bufs=1) as scorep, \
             tc.tile_pool(name="idxp", bufs=5) as idxp, \
             tc.tile_pool(name="gat", bufs=4) as gatp, \
             tc.tile_pool(name="ctp", bufs=2) as ctpool, \
             tc.tile_pool(name="dmy", bufs=2) as dmyp:
            # Packed-score buffers: double-buffered per quarter-row position
            # j (8 total); the lo-u16 iota (global column index) is pre-
            # filled once by a partition-broadcast DMA, ACT refills only the
            # strided hi-u16 f16 score slots each row-tile.
            scbs = [[scorep.tile([128, 1024], u32, tag=f"sc{j}p{p}",
                                 name=f"sc{j}p{p}")
                     for j in range(4)] for p in range(2)]
            iap = iota_d.ap()
            for p in range(2):
                for j in range(4):
                    nc.sync.dma_start(
                        scbs[p][j][:],
                        bass.AP(iap.tensor, iap.offset + j * 1024,
                                [[0, 128], [1, 1024]]))

            def emit_scores(m):
                """Emit packed scores for row-tile m into scbs[m%2][0..3]."""
                for j in range(4):
                    ps = scps.tile([128, 1024], f32, space="PSUM", tag="sc")
                    for h in range(2):
                        nc.tensor.matmul(
                            ps[:, h * 512:(h + 1) * 512],
                            lhsT=lhs_cat[:, bass.ts(m, 128)],
                            rhs=rhs_cat[:, j * 1024 + h * 512:
                                        j * 1024 + (h + 1) * 512],
                            start=True, stop=True)
                    # small negative bias keeps every packed word out of the
                    # f32-denormal range (max8 FTZ would corrupt the index
                    # bits) while preserving f16 resolution at the
                    # near-neighbor distances that decide the top-k
                    hi16 = scbs[m % 2][j][:].bitcast(f16)
                    nc.scalar.activation(
                        bass.AP(hi16.tensor, hi16.offset + 1,
                                [list(hi16.ap[0]), [2, 1024]]),
                        ps[:], ACTF.Copy, bias=-1.0 / 64)

            # Tiles are processed in groups of GRP: the per-tile index
            # scatters land in one per-group DRAM tile, and a single
            # readback DMA pair per group builds the wrapped index lists
            # for all GRP gathers -- amortizing the ~25us DMA completion
            # latency that would otherwise sit in every tile's chain.
            GRP = 4
            NG = NM // GRP
            idxdgs = []
            idxwgs = []

            def topk_and_scatter(m):
                """Top-20 of tile m's packed scores -> parity-split index
                scatter into the group's DRAM tile."""
                # candidate pass: per-256-chunk max8 on the f32 view of the
                # packed words == top-8 (score, index) pairs per chunk
                pk32 = idxp.tile([128, 16 * 8], u32, tag="pk")
                pkf = pk32[:].bitcast(f32)
                for j in range(4):
                    scf = scbs[m % 2][j][:].bitcast(f32)
                    for cc in range(4):
                        c = j * 4 + cc
                        nc.vector.max(out=pkf[:, c * 8:(c + 1) * 8],
                                      in_=scf[:, cc * 256:(cc + 1) * 256])
                # three max8 rounds on the packed candidates -> top-24
                vals24 = idxp.tile([128, 24], f32, tag="v24")
                for r in range(3):
                    nc.vector.max(out=vals24[:, r * 8:(r + 1) * 8], in_=pkf)
                    if r < 2:
                        nc.vector.match_replace(
                            out=pkf, in_to_replace=vals24[:, r * 8:(r + 1) * 8],
                            in_values=pkf, imm_value=-1.0e38)
                v16 = vals24[:].bitcast(i16)
                # bounce through DRAM to build the per-core wrapped index
                # lists: cores 0-3 (partitions 0:64) gather the even-rank
                # neighbors, cores 4-7 the odd ranks.  Gather column order is
                # i = 160a + 16jj + r for point p = 16a + r, so the scatter
                # writes contiguous 10-element runs (idxd[r, 10a+jj]) instead
                # of 2-byte singles; read back 4x-replicated per core.
                idx16e = idxp.tile([128, NJ2], i16, tag="idx16e")
                idx16o = idxp.tile([128, NJ2], i16, tag="idx16o")
                for par, it in enumerate((idx16e, idx16o)):
                    nc.vector.tensor_copy(
                        out=it[:],
                        in_=bass.AP(v16.tensor, v16.offset + 2 * par,
                                    [list(v16.ap[0]), [4, NJ2]]))
                if m % GRP == 0:
                    idxdgs.append(dramp.tile([16, GRP * 160], i16,
                                             tag="idxdg", name="idxdg"))
                dap = idxdgs[m // GRP][:]
                for par, it in enumerate((idx16e, idx16o)):
                    nc.sync.dma_start(
                        bass.AP(dap.tensor,
                                dap.offset + 320 * par + 80 * (m % GRP),
                                [[NJ2, 8], [GRP * 160, 16], [1, NJ2]]),
                        it[:])

            def readback(g):
                """DRAM -> wrapped 4x-replicated per-core index lists for
                all GRP tiles of group g (even half -> partitions 0:64)."""
                idxwg = idxp.tile([128, GRP * 80], i16, tag="idxwg")
                dap = idxdgs[g][:]
                for par in range(2):
                    nc.sync.dma_start(
                        idxwg[64 * par:64 * (par + 1), :],
                        bass.AP(dap.tensor, dap.offset + 320 * par,
                                [[0, 4], [GRP * 160, 16], [1, GRP * 80]]))
                idxwgs.append(idxwg)

            emit_scores(0)

            # A-table build: At[ch, p] duplicated across both partition
            # halves by the [w1a|w1a] weight; value lands in the lo u16 of
            # each u32 word (hi u16 is never read downstream).
            with tc.tile_pool(name="tbps", bufs=2, space="PSUM") as tbps:
                tab16 = tab32[:].bitcast(f16)
                for j8 in range(8):
                    ps = tbps.tile([128, 512], f32, space="PSUM", tag="tb")
                    nc.tensor.matmul(ps[:],
                                     lhsT=w1at2_sb[:],
                                     rhs=lhs_cat[0:3, bass.ts(j8, 512)],
                                     start=True, stop=True)
                    nc.scalar.copy(
                        bass.AP(tab16.tensor, tab16.offset + j8 * 1024,
                                [list(tab16.ap[0]), [2, 512]]),
                        ps[:])

            def do_tile(m):
                # SBUF gather through the Q7 TIE queues: each core pulls its
                # 1280 neighbor columns of the u32 A-table for its own 16
                # channel-partitions.  No DMA descriptors involved.
                gat32 = gatp.tile([128, GNI], u32, tag="gat")
                nc.gpsimd.ap_gather(
                    out_ap=gat32[:], in_ap=tab32[:],
                    idxs_ap=idxwgs[m // GRP][:, 80 * (m % GRP):
                                             80 * (m % GRP) + 80],
                    channels=128, num_elems=N, d=1, num_idxs=GNI)
                # C^T for this tile, duplicated to both partition halves via
                # the duplicated-column w1ct2 weight
                cps = ctps.tile([128, 128], f32, space="PSUM", tag="ct")
                nc.tensor.matmul(cps[:],
                                 lhsT=w1ct2_sb[:],
                                 rhs=lhs_cat[0:3, bass.ts(m, 128)],
                                 start=True, stop=True)
                ct_t = ctpool.tile([128, 128], f16, tag="ct")
                nc.scalar.copy(ct_t[:], cps[:])
                # h1_pre = A_nbr + C_center: even-rank neighbors ->
                # partitions 0:64, odd -> 64:128; gathered values sit at
                # stride 2 (lo u16 halves).  Column i = 160a + 16jj + r is
                # point p = 16a + r, neighbor rank 2jj+par.
                mcol = m * GNI
                ge = gat32[0:CH, :].bitcast(f16)
                go = gat32[CH:128, :].bitcast(f16)
                ce = ct_t[0:CH, :]
                co = ct_t[CH:128, :]
                nc.vector.tensor_tensor(
                    out=_redim(h1p[0:CH, :], [[160, 8], [16, NJ2], [1, 16]],
                               extra_off=mcol),
                    in0=_redim(ge, [[320, 8], [32, NJ2], [2, 16]]),
                    in1=_redim(ce, [[16, 8], [0, NJ2], [1, 16]]),
                    op=ALU.add)
                nc.vector.tensor_tensor(
                    out=_redim(h1p[CH:128, :], [[160, 8], [16, NJ2], [1, 16]],
                               extra_off=mcol),
                    in0=_redim(go, [[320, 8], [32, NJ2], [2, 16]]),
                    in1=_redim(co, [[16, 8], [0, NJ2], [1, 16]]),
                    op=ALU.add)
                # BN1 stats (sum / sumsq per channel) via ACT accumulators
                dmy1 = dmyp.tile([128, GNI], f16, tag="dmy1")
                nc.scalar.activation(dmy1[:], h1p[:, mcol:mcol + GNI],
                                     ACTF.Copy,
                                     accum_out=s1sum[:, m:m + 1])
                dmy2 = dmyp.tile([128, GNI], f16, tag="dmy2")
                nc.scalar.activation(dmy2[:], h1p[:, mcol:mcol + GNI],
                                     ACTF.Square,
                                     accum_out=s1sq[:, m:m + 1])

            # group-level software pipeline: group-iteration k issues the
            # readback for group k+1 (whose scatters completed last
            # iteration), the top-k + scatters for group k+2, and the
            # gathers + adds + stats for group k (whose readback completed
            # last iteration).
            for t in range(2 * GRP):
                if t > 0:
                    emit_scores(t)
                topk_and_scatter(t)
            readback(0)
            for k in range(NG):
                if k + 1 < NG:
                    readback(k + 1)
                if k + 2 < NG:
                    for t in range((k + 2) * GRP, (k + 3) * GRP):
                        emit_scores(t)
                        topk_and_scatter(t)
                for m in range(k * GRP, (k + 1) * GRP):
                    do_tile(m)

        # ------------------------------------------------------------------
        # BN1: global stats -> a1, b1
        # ------------------------------------------------------------------
        with nc.named_scope("bn1"), tc.tile_pool(name="bn1p", bufs=1) as bnp:
            st1 = bnp.tile([128, 2], f32)
            nc.vector.tensor_reduce(out=st1[:, 0:1], in_=s1sum[:],
                                    axis=mybir.AxisListType.X,
                                    op=ALU.add)
            nc.vector.tensor_reduce(out=st1[:, 1:2], in_=s1sq[:],
                                    axis=mybir.AxisListType.X,
                                    op=ALU.add)
            nc.sync.dma_start(cc1_in.ap(), st1[:])
            nc.gpsimd.collective_compute(
                kind="AllReduce", op=ALU.add,
                replica_groups=groups, ins=[cc1_in.ap()], outs=[cc1_out.ap()])
            st1g = bnp.tile([128, 2], f32)
            nc.sync.dma_start(st1g[:], cc1_out.ap())
            st1hi = bnp.tile([CH, 2], f32)
            nc.sync.dma_start(st1hi[:], st1g[CH:128, :])
            tot1 = bnp.tile([CH, 2], f32)
            nc.vector.tensor_add(tot1[:], st1g[0:CH, :], st1hi[:])
            mex = bnp.tile([CH, 2], f32)
            nc.scalar.mul(mex[:], tot1[:], 1.0 / CNT)
            mean1 = mex[:, 0:1]
            msq = bnp.tile([CH, 1], f32)
            nc.scalar.square(msq[:], mean1)
            var1 = bnp.tile([CH, 1], f32)
            nc.vector.tensor_sub(var1[:], mex[:, 1:2], msq[:])
            nc.scalar.activation(var1[:], var1[:], ACTF.Copy, bias=EPS)
            rcp1 = bnp.tile([CH, 1], f32)
            nc.vector.reciprocal(rcp1[:], var1[:])
            rs1 = bnp.tile([CH, 1], f32)
            nc.scalar.sqrt(rs1[:], rcp1[:])
            a1h = bnp.tile([CH, 1], f32)
            nc.vector.tensor_mul(a1h[:], rs1[:], g1_sb[:])
            am = bnp.tile([CH, 1], f32)
            nc.vector.tensor_mul(am[:], a1h[:], mean1)
            b1h = bnp.tile([CH, 1], f32)
            nc.vector.tensor_sub(b1h[:], b1in_sb[:], am[:])
            nc.sync.dma_start(a1_sb[0:CH, :], a1h[:])
            nc.sync.dma_start(a1_sb[CH:128, :], a1h[:])
            nc.sync.dma_start(b1_sb[0:CH, :], b1h[:])
            nc.sync.dma_start(b1_sb[CH:128, :], b1h[:])

        # ------------------------------------------------------------------
        # Phase C: h2_pre = W2blk @ lrelu(a1*h1_pre + b1); BN2 stats; max-pool
        # ------------------------------------------------------------------
        abpool.release()
        cdpool = tc.alloc_tile_pool(name="cd", bufs=1)
        pooled_all = cdpool.tile([CH, N], f32)
        pm_all = cdpool.tile([128, N], f32)    # parity-stacked maxima
        s1a = cdpool.tile([128, NM], f32)      # per-tile sum of h1a
        s2sq = cdpool.tile([128, 3 * NM], f32)  # per-chunk sumsq of h2
        with nc.named_scope("phaseC"), \
             tc.tile_pool(name="h1a", bufs=3) as h1ap, \
             tc.tile_pool(name="pmx", bufs=1) as pmxp, \
             tc.tile_pool(name="dmy2", bufs=4) as dmy2p, \
             tc.tile_pool(name="h2ps", bufs=2, space="PSUM") as h2ps:
            def prelu(m):
                h1a = h1ap.tile([128, GNI], f16, tag="h1a")
                nc.scalar.activation(h1a[:], h1p[:, m * GNI:(m + 1) * GNI],
                                     ACTF.Prelu, bias=b1_sb[:, 0:1],
                                     scale=a1_sb[:, 0:1], alpha=ALPHA,
                                     accum_out=s1a[:, m:m + 1])
                return h1a

            h1as = [prelu(0)]
            for m in range(NM):
                if m + 1 < NM:
                    h1as.append(prelu(m + 1))
                h1a = h1as[m]
                # block-diagonal W2 computes both parities in one matmul
                hp = h2ps.tile([128, GNI], f32, space="PSUM", tag="h2")
                for ci, (c0, c1) in enumerate(
                        ((0, 512), (512, 1024), (1024, 1280))):
                    nc.tensor.matmul(
                        hp[:, c0:c1],
                        lhsT=w2t_sb[:],
                        rhs=h1a[:, c0:c1],
                        start=True, stop=True)
                    scol = m * 3 + ci
                    dmy = dmy2p.tile([128, 512], f16, tag="dmy")
                    nc.scalar.activation(dmy[:, 0:c1 - c0], hp[:, c0:c1],
                                         ACTF.Square,
                                         accum_out=s2sq[:, scol:scol + 1])
                # max over the NJ2 rank-pairs: column i = 160a + 16jj + r
                hb = hp[:]
                rm_in = bass.AP(hb.tensor, hb.offset,
                                [list(hb.ap[0]), [160, 8], [1, 16],
                                 [16, NJ2]])
                pa = pm_all[:, bass.ts(m, 128)]
                pmo = bass.AP(pa.tensor, pa.offset,
                              [list(pa.ap[0]), [16, 8], [1, 16]])
                nc.vector.tensor_reduce(
                    out=pmo, in_=rm_in,
                    axis=mybir.AxisListType.X, op=ALU.max)
            # single cross-parity fold over all tiles at once
            pmf_all = pmxp.tile([CH, N], f32)
            nc.sync.dma_start(pmf_all[:], pm_all[CH:128, :])
            nc.vector.tensor_tensor(
                out=pooled_all[:], in0=pm_all[0:CH, :],
                in1=pmf_all[:], op=ALU.max)

        # ------------------------------------------------------------------
        # BN2: aggregate + global stats -> a2, b2
        # ------------------------------------------------------------------
        with nc.named_scope("bn2"), \
             tc.tile_pool(name="bn2p", bufs=1) as bnp, \
             tc.tile_pool(name="bn2ps", bufs=1, space="PSUM") as bnps:
            s1at = bnp.tile([128, 1], f32)
            nc.vector.tensor_reduce(out=s1at[:], in_=s1a[:],
                                    axis=mybir.AxisListType.X,
                                    op=ALU.add)
            s1a16 = bnp.tile([128, 1], f16)
            nc.vector.tensor_copy(s1a16[:], s1at[:])
            smp = bnps.tile([128, 512], f32, space="PSUM")
            nc.tensor.matmul(smp[:, 0:1], lhsT=w2t_sb[:], rhs=s1a16[:],
                             start=True, stop=True)
            st2p = bnp.tile([128, 2], f32)
            nc.scalar.copy(st2p[:, 0:1], smp[:, 0:1])
            nc.vector.tensor_reduce(out=st2p[:, 1:2], in_=s2sq[:],
                                    axis=mybir.AxisListType.X,
                                    op=ALU.add)
            st2hi = bnp.tile([CH, 2], f32)
            nc.sync.dma_start(st2hi[:], st2p[CH:128, :])
            st2 = bnp.tile([CH, 2], f32)
            nc.vector.tensor_add(st2[:], st2p[0:CH, :], st2hi[:])
            nc.sync.dma_start(cc2_in.ap(), st2[:])
            nc.gpsimd.collective_compute(
                kind="AllReduce", op=ALU.add,
                replica_groups=groups, ins=[cc2_in.ap()], outs=[cc2_out.ap()])
            tot2 = bnp.tile([CH, 2], f32)
            nc.sync.dma_start(tot2[:], cc2_out.ap())
            mex2 = bnp.tile([CH, 2], f32)
            nc.scalar.mul(mex2[:], tot2[:], 1.0 / CNT)
            mean2 = mex2[:, 0:1]
            msq2 = bnp.tile([CH, 1], f32)
            nc.scalar.square(msq2[:], mean2)
            var2 = bnp.tile([CH, 1], f32)
            nc.vector.tensor_sub(var2[:], mex2[:, 1:2], msq2[:])
            nc.scalar.activation(var2[:], var2[:], ACTF.Copy, bias=EPS)
            rcp2 = bnp.tile([CH, 1], f32)
            nc.vector.reciprocal(rcp2[:], var2[:])
            rs2 = bnp.tile([CH, 1], f32)
            nc.scalar.sqrt(rs2[:], rcp2[:])
            nc.vector.tensor_mul(a2_sb[:], rs2[:], g2_sb[:])
            am2 = bnp.tile([CH, 1], f32)
            nc.vector.tensor_mul(am2[:], a2_sb[:], mean2)
            nc.vector.tensor_sub(b2_sb[:], b2in_sb[:], am2[:])

        # ------------------------------------------------------------------
        # Phase D: apply BN2 + lrelu to the pooled maxima (BN2 scale >= 0 and
        # lrelu monotone, so max commutes past them)
        # ------------------------------------------------------------------
        with nc.named_scope("phaseD"), \
             tc.tile_pool(name="runm", bufs=4) as runp:
            for m in range(NM):
                runmax = runp.tile([CH, 128], f32, tag="run")
                nc.scalar.activation(runmax[:], pooled_all[:, bass.ts(m, 128)],
                                     ACTF.Prelu, bias=b2_sb[:, 0:1],
                                     scale=a2_sb[:, 0:1], alpha=ALPHA)
                nc.sync.dma_start(out_t.ap()[:, bass.ts(m, 128)], runmax[:])

        cdpool.release()
        const.release()
        dramp.release()

    # Raw Bass skips Bacc's codegen_inst_isa_subclasses; without it the
    # library-reload pseudo-instruction serializes with empty .instr bytes
    # and walrus fails with "ISA wrong length".
    mybir.codegen_inst_isa_subclasses(nc)
    return nc


_prog_cache = {}


def _get_program():
    if "nc" not in _prog_cache:
        _prog_cache["nc"] = _build_program()
    return _prog_cache["nc"]


def make_in_maps(x, W1, gamma1, beta1, W2, gamma2, beta2):
    x = np.asarray(x, dtype=np.float32)
    W1 = np.asarray(W1, dtype=np.float32)
    W2 = np.asarray(W2, dtype=np.float32)
    w1a = np.ascontiguousarray(W1[:, 0:3].T).astype(np.float16)   # [3, 64]
    w1at2 = np.concatenate([w1a, w1a], axis=1)                    # [3, 128]
    w1c = np.ascontiguousarray((W1[:, 3:6] - W1[:, 0:3]).T).astype(np.float16)
    w1ct2 = np.concatenate([w1c, w1c], axis=1)                    # [3, 128]
    g2 = np.asarray(gamma2, dtype=np.float32).reshape(CH)
    sgn2 = np.where(g2 < 0, -1.0, 1.0).astype(np.float32)
    W2f = W2 * sgn2[:, None]          # flip rows so the BN2 scale is >= 0
    w2tb = np.ascontiguousarray(W2f.T).astype(np.float16)     # [64, 64]
    z = np.zeros_like(w2tb)
    w2t = np.block([[w2tb, z], [z, w2tb]])                    # [128, 128]
    iota = np.arange(N, dtype=np.uint32).reshape(1, N)
    col = lambda v: np.ascontiguousarray(
        np.asarray(v, dtype=np.float32).reshape(CH, 1))
    return [{
        "xb": np.concatenate([x[b], np.ones((1, N), np.float32)], axis=0),
        "w1at2": w1at2, "w1ct2": w1ct2, "w2t": w2t, "iota": iota,
        "bn1g": col(gamma1), "bn1b": col(beta1),
        "bn2g": col(np.abs(g2)), "bn2b": col(beta2),
    } for b in range(B)]


def kernel(x, W1, gamma1, beta1, W2, gamma2, beta2):
    nc = _get_program()
    in_maps = make_in_maps(x, W1, gamma1, beta1, W2, gamma2, beta2)
    res = run_bass_kernel_spmd(nc, in_maps, list(range(B)))
    out = np.stack([res.results[b]["out"] for b in range(B)], axis=0)
    return out.astype(np.float32)
